# revision 1
# baseline (speedup 1.0000x reference)
"""Trainium2 Bass kernel for nn_AttentionLayer (linear attention, sparse_attention).

Math (per batch element n of B*H*W=2304):
    q = qin @ Wq + bq                (N=80 tokens, C=256 -> 128)
    k = [x|guidance] @ Wk + bk       (S=77 tokens)
    v = x @ Wv + bv
    Q = elu(q)+1, K = elu(k)+1       (8 heads x 16 dim)
    ZD[l,h]   = sum_d Q[l,hd]*Ksum[hd],  Ksum = sum_s K
    Z         = 1/(ZD+eps)
    Qbar[hd]  = sum_l Q[l,hd]*Z[l,h]
    A[h,s]    = sum_d Qbar[hd]*K[s,hd]
    out[hv]   = (1/N) * sum_s A[h,s]*v[s,hv]     (exact refactor of reference)

Everything on-chip lives in the "transposed" basis: feature dim (128) on SBUF
partitions, tokens on the free axis.  The host supplies pre-transposed bf16
inputs so every DMA is contiguous, and transposes the (128, n) output back.
"""

import numpy as np
import ml_dtypes

import concourse.bass as bass
import concourse.bacc as bacc
import concourse.mybir as mybir
import concourse.tile as tile
from concourse.bass_utils import run_bass_kernel_spmd

F32 = mybir.dt.float32
BF16 = mybir.dt.bfloat16
AF = mybir.ActivationFunctionType
ALU = mybir.AluOpType



NCORES = 8
NH, DH, HID = 8, 16, 128
S, NTOK, C = 77, 80, 256
B, N_, H_, W_ = 4, 80, 24, 24
NTOTAL = B * H_ * W_            # 2304
NLOC = NTOTAL // NCORES         # 288
GRP = 6                         # batch elements per group
NGRP_SUPER = 4                  # groups per supergroup (packs 4x8 rows into PSUM)
SUPER = GRP * NGRP_SUPER        # 24 n per supergroup
EPS = 1e-6


def build_nc(n_loc: int = NLOC) -> bass.Bass:
    assert n_loc % SUPER == 0
    nsuper = n_loc // SUPER

    nc = bacc.Bacc()

    qt = nc.declare_dram_parameter("qt", [2, HID, n_loc * NTOK], BF16, isOutput=False)
    xt = nc.declare_dram_parameter("xt", [HID, n_loc * S], BF16, isOutput=False)
    gt = nc.declare_dram_parameter("gt", [HID, n_loc * S], BF16, isOutput=False)
    CBW = 5 * HID + HID + 2 * GRP * NH + 512 + 10 + 640  # ...|zeros|f32bits|ones128|eps512
    cbp = nc.declare_dram_parameter("cb", [HID, CBW], BF16, isOutput=False)
    out = nc.declare_dram_parameter("o", [HID, n_loc], F32, isOutput=True)

    QF = GRP * NTOK   # 480 free elems per group (q side)
    KF = GRP * S      # 462 free elems per group (k/v side)
    QS = SUPER * NTOK  # 1920 per supergroup
    KS = SUPER * S     # 1848

    with tile.TileContext(nc) as tc:
        with (
            tc.tile_pool(name="consts", bufs=1) as consts,
            tc.tile_pool(name="dmain", bufs=2) as dmain,
            tc.tile_pool(name="work", bufs=3) as work,
            tc.tile_pool(name="persist", bufs=6) as persist,
            tc.tile_pool(name="za", bufs=2) as za,
            tc.tile_pool(name="small", bufs=4) as small,
            tc.tile_pool(name="outp", bufs=1) as outp,
            tc.tile_pool(name="pq", bufs=1, space="PSUM") as pqp,
            tc.tile_pool(name="pk", bufs=1, space="PSUM") as pkp,
            tc.tile_pool(name="pv", bufs=1, space="PSUM") as pvp,
            tc.tile_pool(name="pzae", bufs=2, space="PSUM") as pzaep,
            tc.tile_pool(name="pzd", bufs=2, space="PSUM") as pzdp,
            tc.tile_pool(name="pa", bufs=1, space="PSUM") as pap,
        ):
            # ---- constants: two packed blobs, two DMAs ----
            cb_t = consts.tile([HID, CBW], BF16)
            nc.sync.dma_start(cb_t[:], cbp[:])
            fb_t = cb_t[:, CBW - 650:CBW - 640].bitcast(F32)
            ones_t = cb_t[0:1, CBW - 640:CBW - 512]
            epsr_t = cb_t[0:1, CBW - 512:CBW]
            wq0_t = cb_t[:, 0:HID]
            wq1_t = cb_t[:, HID:2 * HID]
            wk0_t = cb_t[:, 2 * HID:3 * HID]
            wk1_t = cb_t[:, 3 * HID:4 * HID]
            wv_t = cb_t[:, 4 * HID:5 * HID]
            e8_t = cb_t[:, 5 * HID:6 * HID]
            m1_t = cb_t[:, 6 * HID:6 * HID + GRP * NH]
            mn_t = cb_t[:, 6 * HID + GRP * NH:6 * HID + 2 * GRP * NH]
            zr_t = cb_t[0:1, 6 * HID + 2 * GRP * NH:6 * HID + 2 * GRP * NH + 512]
            bq_t = fb_t[:, 0:1]
            bk_t = fb_t[:, 1:2]
            bv_t = fb_t[:, 2:3]
            eps_t = fb_t[:, 3:4]
            zero_t = fb_t[:, 4:5]
            tc.strict_bb_all_engine_barrier()

            outT = outp.tile([HID, n_loc], F32)

            for sg in range(nsuper):
                # ---- supergroup DMA in ----
                qt_sb = dmain.tile([HID, 2, QS], BF16, tag="qt")
                xt_sb = dmain.tile([HID, KS], BF16, tag="xt")
                gt_sb = dmain.tile([HID, KS], BF16, tag="gt")
                for po in range(2):
                    nc.sync.dma_start(
                        qt_sb[:, po, :], qt[po, :, sg * QS:(sg + 1) * QS]
                    )
                nc.sync.dma_start(xt_sb[:], xt[:, sg * KS:(sg + 1) * KS])
                nc.sync.dma_start(gt_sb[:], gt[:, sg * KS:(sg + 1) * KS])

                # supergroup-lifetime PSUM banks (packed 8-row slots x 4 groups)
                pzd = pzdp.tile([HID, 512], F32, tag="pzd")
                pzd = pzd[:, :QF]
                pa = pap.tile([HID, 512], F32, tag="pa")
                pa = pa[:, :KF]
                # zero-init so ACT reads of untouched rows are defined
                nc.tensor.matmul(pzd[:], ones_t[:], epsr_t[:, :QF],
                                 start=True, stop=True)
                nc.tensor.matmul(pa[:], zr_t[:, :HID], zr_t[:, :KF],
                                 start=True, stop=True)

                kfm_g = []
                qfm_g = []
                vsb_g = []
                # ================= front half: proj + fm + ZD =================
                for g in range(NGRP_SUPER):
                    qs = slice(g * QF, (g + 1) * QF)
                    ks = slice(g * KF, (g + 1) * KF)

                    pq = pqp.tile([HID, 512], F32, tag="pq")
                    pq = pq[:, :QF]
                    pk = pkp.tile([HID, 512], F32, tag="pk")
                    pk = pk[:, :KF]
                    pv = pvp.tile([HID, 512], F32, tag="pv")
                    pv = pv[:, :KF]
                    nc.tensor.matmul(pq[:], wq0_t, qt_sb[:, 0, qs],
                                     start=True, stop=False)
                    nc.tensor.matmul(pq[:], wq1_t, qt_sb[:, 1, qs],
                                     start=False, stop=True)
                    nc.tensor.matmul(pk[:], wk0_t, xt_sb[:, ks],
                                     start=True, stop=False)
                    nc.tensor.matmul(pk[:], wk1_t, gt_sb[:, ks],
                                     start=False, stop=True)
                    nc.tensor.matmul(pv[:], wv_t, xt_sb[:, ks],
                                     start=True, stop=True)

                    # feature map: fm(y) = min(exp(y), 1 + relu(y))
                    eq = work.tile([HID, QF], BF16, tag="eq")
                    rq = work.tile([HID, QF], BF16, tag="rq")
                    qfm = persist.tile([HID, QF], BF16, tag="qfm")
                    ek = work.tile([HID, KF], BF16, tag="ek")
                    rk = work.tile([HID, KF], BF16, tag="rk")
                    kfm = persist.tile([HID, KF], BF16, tag="kfm")
                    vsb = persist.tile([HID, KF], BF16, tag="vsb")
                    nc.scalar.activation(eq[:], pq[:], AF.Exp, bias=bq_t)
                    nc.scalar.activation(rq[:], pq[:], AF.Relu, bias=bq_t)
                    nc.scalar.activation(ek[:], pk[:], AF.Exp, bias=bk_t)
                    nc.scalar.activation(rk[:], pk[:], AF.Relu, bias=bk_t)
                    nc.scalar.activation(vsb[:], pv[:], AF.Identity, bias=bv_t)
                    eqm = work.tile([HID, QF], BF16, tag="eqm")
                    ekm = work.tile([HID, KF], BF16, tag="ekm")
                    nc.vector.tensor_scalar_min(eqm[:], eq[:], 1.0)
                    nc.vector.tensor_scalar_min(ekm[:], ek[:], 1.0)
                    nc.gpsimd.tensor_tensor(qfm[:], eqm[:], rq[:], ALU.add)
                    nc.gpsimd.tensor_tensor(kfm[:], ekm[:], rk[:], ALU.add)

                    # Ksum (128, GRP) then KBD = mask1 * Ksum  (8 cols per n)
                    ksum = small.tile([HID, GRP], F32, tag="ksum")
                    kbd = small.tile([HID, GRP * NH], BF16, tag="kbd")
                    nc.vector.tensor_reduce(
                        ksum[:], kfm[:].rearrange("p (g s) -> p g s", s=S),
                        mybir.AxisListType.X, ALU.add)
                    nc.vector.tensor_tensor(
                        kbd[:].rearrange("p (g h) -> p g h", h=NH),
                        m1_t[:].rearrange("p (g h) -> p g h", h=NH),
                        ksum[:, :, None].to_broadcast((HID, GRP, NH)),
                        ALU.mult)

                    # ZD rows for this group -> packed at partition base 32*g
                    for i in range(GRP):
                        nc.tensor.matmul(
                            pzd[32 * g:32 * g + NH, i * NTOK:(i + 1) * NTOK],
                            kbd[:, i * NH:(i + 1) * NH],
                            qfm[:, i * NTOK:(i + 1) * NTOK],
                            start=False, stop=True, skip_group_check=True,
                            tile_position=(0, 32 * g))
                    kfm_g.append(kfm)
                    qfm_g.append(qfm)
                    vsb_g.append(vsb)

                # ========== supergroup: Z = 1/(ZD+eps) (eps baked into init) ==========
                zf = za.tile([HID, QF], F32, tag="zf")
                zpk = za.tile([HID, QF], BF16, tag="zpk")
                nc.vector.reciprocal_approx_fast(zf[:], pzd[:])
                nc.vector.tensor_copy(zpk[:], zf[:])

                # ================= back half =================
                apk = za.tile([HID, KF], BF16, tag="apk")
                for g in range(NGRP_SUPER):
                    qfm = qfm_g[g]
                    kfm = kfm_g[g]
                    vsb = vsb_g[g]
                    rowg = slice(32 * g, 32 * g + NH)

                    # Zexp (128, 480): one expander matmul per group
                    pze = pzaep.tile([HID, 512], F32, tag="pze")
                    pze = pze[:, :QF]
                    nc.tensor.matmul(
                        pze[:], e8_t[rowg, :], zpk[rowg, :],
                        start=True, stop=True, tile_position=(32 * g, 0))

                    # Qbar[hd, n] = sum_l qfm * zexp
                    qbar = small.tile([HID, GRP], F32, tag="qbar")
                    ttr_s = small.tile([HID, NTOK], BF16, tag="ttrs")
                    for i in range(GRP):
                        nc.vector.scalar_tensor_tensor(
                            ttr_s[:],
                            qfm[:, i * NTOK:(i + 1) * NTOK],
                            0.0,
                            pze[:, i * NTOK:(i + 1) * NTOK],
                            ALU.bypass, ALU.mult,
                            accum_out=qbar[:, i:i + 1])

                    # Abd = maskn * Qbar ; A^T rows packed at base 32*g
                    abd = small.tile([HID, GRP * NH], BF16, tag="abd")
                    nc.vector.tensor_tensor(
                        abd[:].rearrange("p (g h) -> p g h", h=NH),
                        mn_t[:].rearrange("p (g h) -> p g h", h=NH),
                        qbar[:, :, None].to_broadcast((HID, GRP, NH)),
                        ALU.mult)
                    for i in range(GRP):
                        nc.tensor.matmul(
                            pa[rowg, i * S:(i + 1) * S],
                            abd[:, i * NH:(i + 1) * NH],
                            kfm[:, i * S:(i + 1) * S],
                            start=False, stop=True, skip_group_check=True,
                            tile_position=(0, 32 * g))

                # A^T -> SBUF bf16 once per supergroup
                nc.scalar.activation(apk[:], pa[:], AF.Copy)

                for g in range(NGRP_SUPER):
                    vsb = vsb_g[g]
                    rowg = slice(32 * g, 32 * g + NH)
                    pae = pzaep.tile([HID, 512], F32, tag="pze")
                    pae = pae[:, :KF]
                    nc.tensor.matmul(
                        pae[:], e8_t[rowg, :], apk[rowg, :],
                        start=True, stop=True, tile_position=(32 * g, 0))
                    # out^T[:, n] = sum_s vsb * aexp
                    stt_s = small.tile([HID, S], BF16, tag="stts")
                    for i in range(GRP):
                        nglob = sg * SUPER + g * GRP + i
                        nc.vector.scalar_tensor_tensor(
                            stt_s[:],
                            vsb[:, i * S:(i + 1) * S],
                            0.0,
                            pae[:, i * S:(i + 1) * S],
                            ALU.bypass, ALU.mult,
                            accum_out=outT[:, nglob:nglob + 1])

            nc.sync.dma_start(out[:], outT[:])

    nc.finalize()
    return nc


# ---------------- host-side packing ----------------

def make_consts():
    hd = np.arange(HID)
    e8 = (hd[None, :] // DH == (np.arange(HID) % NH)[:, None]).astype(
        ml_dtypes.bfloat16)
    m1 = np.zeros((HID, GRP * NH), np.float32)
    for i in range(GRP):
        for h in range(NH):
            m1[h * DH:(h + 1) * DH, i * NH + h] = 1.0
    mn = (m1 / float(NTOK)).astype(np.float32)
    return e8.astype(np.float32), m1, mn


def shard_inputs(query, x, guidance, Wq, bq, Wk, bk, Wv, bv, n_loc=NLOC,
                 ncores=NCORES):
    qin = np.ascontiguousarray(
        query.transpose(0, 2, 3, 1, 4)).reshape(NTOTAL, NTOK, C)
    e8, m1, mn = make_consts()
    bf = ml_dtypes.bfloat16
    wqr = Wq.reshape(2, HID, HID)
    wkr = Wk.reshape(2, HID, HID)
    cb = np.concatenate(
        [wqr[0], wqr[1], wkr[0], wkr[1], Wv, e8.astype(np.float32),
         m1.astype(np.float32), mn.astype(np.float32),
         np.zeros((HID, 512), np.float32)], axis=1).astype(bf)
    fb = np.stack(
        [bq, bk, bv, np.full(HID, 1e-6, np.float32),
         np.zeros(HID, np.float32)], axis=1).astype(np.float32)
    fb_as_bf = np.ascontiguousarray(fb).view(bf)
    extra = np.zeros((HID, 640), np.float32)
    extra[0, :128] = 1.0
    extra[0, 128:] = 1e-6
    cb = np.concatenate([cb, fb_as_bf, extra.astype(bf)], axis=1)
    shared = dict(cb=cb)
    in_maps = []
    for i in range(ncores):
        sl = slice(i * n_loc, (i + 1) * n_loc)
        qc = qin[sl].reshape(n_loc * NTOK, C)
        xc = x[sl].reshape(n_loc * S, HID)
        gc = guidance[sl].reshape(n_loc * S, HID)
        m = dict(shared)
        m["qt"] = np.ascontiguousarray(qc.T).reshape(2, HID, n_loc * NTOK).astype(bf)
        m["xt"] = np.ascontiguousarray(xc.T).astype(bf)
        m["gt"] = np.ascontiguousarray(gc.T).astype(bf)
        in_maps.append(m)
    return in_maps


_NC_CACHE = {}


def kernel(**inputs) -> np.ndarray:
    inputs = {k: np.asarray(v, dtype=np.float32) if np.asarray(v).dtype != np.int32
              else np.asarray(v) for k, v in inputs.items()}
    in_maps = shard_inputs(**inputs)
    if NLOC not in _NC_CACHE:
        _NC_CACHE[NLOC] = build_nc(NLOC)
    nc = _NC_CACHE[NLOC]
    res = run_bass_kernel_spmd(nc, in_maps, core_ids=list(range(NCORES)))
    outs = [np.asarray(res.results[i]["o"]).T for i in range(NCORES)]
    full = np.concatenate(outs, axis=0)  # (2304, 128)
    return full.reshape(B, H_, W_, HID).astype(np.float32)

